# revision 36
# baseline (speedup 1.0000x reference)
"""Multi-head attention + layernorm Bass kernel for Trainium2, 8 cores.

Problem: B=8, S=1024, D=768, H=12 heads x DH=64, key-padding mask, softmax,
output projection, layernorm.  Sharding: pure data parallelism — one batch
element per NeuronCore, no collectives.

Layout strategy (per core, all matmuls in float32r = 1 cyc/row at N>=256):
  - host passes x^T [768, 1024] so the d-contraction lives on partitions.
  - q^T/k^T built pair-of-heads-stacked: psum [128, 1024] = two heads' [64, S].
  - scores^T computed per key-chunk j with two concurrent row-tiled K=64
    matmuls (heads at partitions 0-63 / 64-127).
  - exp via ACT directly from PSUM (+ key mask as the per-partition bias).
  - ctx^T = sum_j v_aug[j]^T-style matmul with a ones column appended to V so
    each head's softmax denominators fall out of the same matmuls (row 64).
  - denominators: DVE reciprocal -> K=1 matmul broadcast -> DVE multiply while
    copying PSUM->SBUF (head B shifted to partitions 64-127).
  - out projection row-tiled over head pairs; layernorm via bn_stats/bn_aggr.
"""

import numpy as np

B, S, D, H, DH = 8, 1024, 768, 12, 64
NPAIR, NQUAD = H // 2, H // 4
SBLK = S // 128      # 8 key/row chunks
DCH = D // 128       # 6 contraction chunks
LN_EPS = 1e-5
NEG_MASK = -30.0

_PROGRAM = None


def _build_program():
    import concourse.bass as bass
    from concourse import bacc
    import concourse.tile as tile
    import concourse.mybir as mybir
    from contextlib import ExitStack

    F32 = mybir.dt.float32
    F32R = mybir.dt.float32r
    AF = mybir.ActivationFunctionType

    nc = bacc.Bacc("TRN2", target_bir_lowering=False)

    xt_d = nc.dram_tensor("xt", [D, S], F32R, kind="ExternalInput")
    wq_d = nc.dram_tensor("wq", [D, D], F32R, kind="ExternalInput")
    wk_d = nc.dram_tensor("wk", [D, D], F32R, kind="ExternalInput")
    wv_d = nc.dram_tensor("wv", [D, NQUAD * 260], F32R, kind="ExternalInput")
    wo_d = nc.dram_tensor("wo", [D, D], F32R, kind="ExternalInput")
    bqk_d = nc.dram_tensor("bqk", [128, 2 * NPAIR], F32, kind="ExternalInput")
    bv_d = nc.dram_tensor("bv", [1, NQUAD * 260], F32, kind="ExternalInput")
    maskb_d = nc.dram_tensor("maskb", [128, SBLK], F32, kind="ExternalInput")
    gamma_d = nc.dram_tensor("gamma", [1, D], F32, kind="ExternalInput")
    beta_d = nc.dram_tensor("beta", [1, D], F32, kind="ExternalInput")
    bo_d = nc.dram_tensor("bo", [1, D], F32, kind="ExternalInput")
    sel_d = nc.dram_tensor("sel", [128, 256], F32R, kind="ExternalInput")
    onesr_d = nc.dram_tensor("onesr", [1, 128], F32R, kind="ExternalInput")
    bor_d = nc.dram_tensor("bor", [1, D], F32R, kind="ExternalInput")
    out_d = nc.dram_tensor("out", [S, D], F32, kind="ExternalOutput")

    with tile.TileContext(nc) as tc, ExitStack() as ctx:
        const = ctx.enter_context(tc.tile_pool(name="const", bufs=1))
        xt_p = ctx.enter_context(tc.tile_pool(name="xt_p", bufs=1))
        w_p = ctx.enter_context(tc.tile_pool(name="w_p", bufs=1))
        qk_p = ctx.enter_context(tc.tile_pool(name="qk_p", bufs=1))
        v_p = ctx.enter_context(tc.tile_pool(name="v_p", bufs=1))
        e_p = ctx.enter_context(tc.tile_pool(name="e_p", bufs=1))
        cx_p = ctx.enter_context(tc.tile_pool(name="cx_p", bufs=1))
        z_p = ctx.enter_context(tc.tile_pool(name="z_p", bufs=1))
        # 8 PSUM banks: "big" = 3 x [128,1024] slots (q/k proj, scores, out
        # proj), "small" = 2 x [128,512] slots (v proj, ctx accum, denom bcast)
        ps = ctx.enter_context(tc.tile_pool(name="ps", bufs=1, space="PSUM"))

        # ---- constants ----
        bqk_t = const.tile([128, 2 * NPAIR], F32)
        nc.sync.dma_start(out=bqk_t, in_=bqk_d[:, :])
        bv_t = const.tile([128, NQUAD * 260], F32)
        nc.sync.dma_start(out=bv_t, in_=bv_d[0:1, :].to_broadcast([128, NQUAD * 260]))
        mask_t = const.tile([128, SBLK], F32)
        nc.sync.dma_start(out=mask_t, in_=maskb_d[:, :])
        gamma_t = const.tile([128, D], F32)
        nc.sync.dma_start(out=gamma_t, in_=gamma_d[0:1, :].to_broadcast([128, D]))
        beta_t = const.tile([128, D], F32)
        nc.sync.dma_start(out=beta_t, in_=beta_d[0:1, :].to_broadcast([128, D]))
        bo_t = const.tile([128, D], F32)
        nc.sync.dma_start(out=bo_t, in_=bo_d[0:1, :].to_broadcast([128, D]))
        sel_t = const.tile([128, 256], F32R)
        nc.sync.dma_start(out=sel_t, in_=sel_d[:, :])
        onesr_t = const.tile([1, 128], F32R)
        nc.sync.dma_start(out=onesr_t, in_=onesr_d[:, :])
        bor_t = const.tile([1, D], F32R)
        nc.sync.dma_start(out=bor_t, in_=bor_d[:, :])
        eps_t = const.tile([128, 1], F32)
        nc.vector.memset(eps_t, LN_EPS)

        xt = []
        for c in range(DCH):
            xt_t = xt_p.tile([128, S], F32R, name=f"xt{c}")
            nc.sync.dma_start(out=xt_t, in_=xt_d[c * 128:(c + 1) * 128, :])
            xt.append(xt_t)

        # ---- v projections, per quad of heads (N=260 keeps f32r fast) ----
        v_sb = {}   # (quad, sblk) -> [128, 260] f32r

        def emit_v_quad(q):
            wv_t = []
            for c in range(DCH):
                wvt = w_p.tile([128, 260], F32R, name="wv_t", bufs=3 * DCH)
                nc.sync.dma_start(
                    out=wvt, in_=wv_d[c * 128:(c + 1) * 128, q * 260:(q + 1) * 260])
                wv_t.append(wvt)
            for s in range(SBLK):
                psv = ps.tile([128, 260], F32, name="psv", tag="proj", bufs=1,
                              padded_shape=[128, 1024])
                for c in range(DCH):
                    nc.tensor.matmul(psv, xt[c][:, s * 128:(s + 1) * 128], wv_t[c],
                                     start=(c == 0), stop=(c == DCH - 1))
                vt = v_p.tile([128, 260], F32R, name="v_sb", bufs=3 * SBLK)
                nc.vector.tensor_add(out=vt, in0=psv,
                                     in1=bv_t[:, q * 260:(q + 1) * 260])
                v_sb[(q, s)] = vt

        # ---- per pair: q/k projections then attention ----
        ctxt = []   # per pair [128, 1024] f32r normalized ctx^T (B shifted)

        for p in range(NPAIR):
            if p % 2 == 0:
                emit_v_quad(p // 2)
            wq_t, wk_t = [], []
            for c in range(DCH):
                wqt = w_p.tile([128, 128], F32R, name="wq_t", bufs=4 * DCH)
                nc.sync.dma_start(
                    out=wqt, in_=wq_d[c * 128:(c + 1) * 128, p * 128:(p + 1) * 128])
                wq_t.append(wqt)
                wkt = w_p.tile([128, 128], F32R, name="wk_t", bufs=4 * DCH)
                nc.sync.dma_start(
                    out=wkt, in_=wk_d[c * 128:(c + 1) * 128, p * 128:(p + 1) * 128])
                wk_t.append(wkt)

            psq = ps.tile([128, S], F32, name="psq", tag="proj", bufs=1)
            for half in range(2):
                for c in range(DCH):
                    nc.tensor.matmul(
                        psq[:, half * 512:(half + 1) * 512], wq_t[c],
                        xt[c][:, half * 512:(half + 1) * 512],
                        start=(c == 0), stop=(c == DCH - 1))
            qt = qk_p.tile([128, S], F32R, name="qt_sb", bufs=3)
            nc.vector.tensor_scalar_add(out=qt, in0=psq, scalar1=bqk_t[:, p:p + 1])

            psk = ps.tile([128, S], F32, name="psk", tag="proj", bufs=1)
            for half in range(2):
                for c in range(DCH):
                    nc.tensor.matmul(
                        psk[:, half * 512:(half + 1) * 512], wk_t[c],
                        xt[c][:, half * 512:(half + 1) * 512],
                        start=(c == 0), stop=(c == DCH - 1))
            kt = qk_p.tile([128, S], F32R, name="kt_sb", bufs=3)
            nc.vector.tensor_scalar_add(out=kt, in0=psk,
                                        scalar1=bqk_t[:, NPAIR + p:NPAIR + p + 1])

            ct = cx_p.tile([128, S], F32R, name="ctxt", bufs=NPAIR)
            ctxt.append(ct)
            q, l0 = divmod(2 * p, 4)

            # softmax denominators for this pair: row 32*(2*idx + iblk) =
            # (head idx, query half iblk); engine writes need 32-aligned
            # partition bases.  Unused partitions memset to 1.0 so the
            # batched reciprocal stays finite.
            rpk = z_p.tile([128, 512], F32, name="rpk", bufs=3)
            nc.vector.memset(rpk, 1.0)
            for iblk in range(2):
                pcx = [ps.tile([65, 512], F32, name="pscx", tag="cx", bufs=2)
                       for _ in range(2)]
                for j in range(SBLK):
                    pst = ps.tile([128, 1024], F32, name="psst", tag="st",
                                  bufs=2)
                    nc.tensor.matmul(
                        pst[:, 0:512], kt[0:64, j * 128:(j + 1) * 128],
                        qt[0:64, iblk * 512:(iblk + 1) * 512],
                        start=True, stop=True, tile_position=(0, 0))
                    nc.tensor.matmul(
                        pst[:, 512:1024], kt[64:128, j * 128:(j + 1) * 128],
                        qt[64:128, iblk * 512:(iblk + 1) * 512],
                        start=True, stop=True, tile_position=(64, 0))
                    et = e_p.tile([128, 1024], F32R, name="expt", bufs=3)
                    nc.scalar.activation(et, pst, AF.Exp, bias=mask_t[:, j:j + 1])
                    for idx in range(2):
                        vsl = v_sb[(q, j)][:, (l0 + idx) * 65:(l0 + idx + 1) * 65]
                        nc.tensor.matmul(pcx[idx], vsl,
                                         et[:, idx * 512:(idx + 1) * 512],
                                         start=(j == 0), stop=(j == SBLK - 1))
                # move ctx (rows 0-63) and denominators (row 64) out of PSUM
                for idx in range(2):
                    u = 32 * (2 * idx + iblk)
                    nc.scalar.copy(out=rpk[u:u + 1, :],
                                   in_=pcx[idx][64:65, :])
                    nc.vector.tensor_copy(
                        out=ct[idx * 64:(idx + 1) * 64,
                               iblk * 512:(iblk + 1) * 512],
                        in_=pcx[idx][0:64, :])
            # one batched reciprocal per pair, then one-hot-selector matmuls
            # broadcast each row to [64, 512] and normalize in place
            rinv_p = z_p.tile([128, 512], F32R, name="rinv_p", bufs=2)
            with nc.allow_low_precision(reason="f32r softmax denom"):
                nc.vector.reciprocal(out=rinv_p, in_=rpk)
            for idx in range(2):
                for iblk in range(2):
                    u = 2 * idx + iblk
                    pbc = ps.tile([64, 512], F32, name="psbc", tag="cx",
                                  bufs=2)
                    nc.tensor.matmul(pbc, sel_t[:, u * 64:(u + 1) * 64],
                                     rinv_p, start=True, stop=True)
                    csl = ct[idx * 64:(idx + 1) * 64,
                             iblk * 512:(iblk + 1) * 512]
                    nc.vector.tensor_mul(out=csl, in0=csl, in1=pbc)

        # ---- output projection + layernorm, per row block ----
        # Wo loaded here so its DMAs don't compete with startup traffic
        wo_t = []
        for c in range(DCH):
            wot = w_p.tile([128, D], F32R, name=f"wo{c}")
            nc.sync.dma_start(out=wot, in_=wo_d[c * 128:(c + 1) * 128, :])
            wo_t.append(wot)

        for s in range(SBLK):
            pso = ps.tile([128, D], F32, name="pso", tag="st", bufs=2,
                          padded_shape=[128, 1024])
            for d0, d1 in ((0, 512), (512, 768)):
                for p in range(NPAIR):
                    nc.tensor.matmul(
                        pso[:, d0:d1],
                        ctxt[p][:, s * 128:(s + 1) * 128],
                        wo_t[p][:, d0:d1],
                        start=(p == 0), stop=False)
                # + bo via a K=1 rank-one update: ones_col x bo_row
                nc.tensor.matmul(pso[:, d0:d1], onesr_t, bor_t[:, d0:d1],
                                 start=False, stop=True)
            stats = z_p.tile([128, 3, 6], F32, name="stats", bufs=2)
            for g in range(3):
                nc.vector.bn_stats(out=stats[:, g, :],
                                   in_=pso[:, g * 256:(g + 1) * 256])
            mv = z_p.tile([128, 2], F32, name="mv", bufs=2)
            nc.vector.bn_aggr(out=mv, in_=stats)
            stdv = z_p.tile([128, 1], F32, name="stdv", bufs=2)
            nc.scalar.activation(stdv, mv[:, 1:2], AF.Sqrt, bias=eps_t)
            rstd = z_p.tile([128, 1], F32, name="rstd", bufs=2)
            nc.vector.reciprocal(out=rstd, in_=stdv)
            z = z_p.tile([128, D], F32, name="z_sb", bufs=2)
            nc.vector.tensor_scalar(out=z, in0=pso, scalar1=mv[:, 0:1],
                                    scalar2=rstd, op0=mybir.AluOpType.subtract,
                                    op1=mybir.AluOpType.mult)
            nc.vector.tensor_mul(out=z, in0=z, in1=gamma_t)
            nc.vector.tensor_add(out=z, in0=z, in1=beta_t)
            nc.sync.dma_start(out=out_d[s * 128:(s + 1) * 128, :], in_=z)

    nc.compile()
    return nc


def _host_inputs(inputs):
    x = np.asarray(inputs["input_tensor"], np.float32)
    mask = np.asarray(inputs["attention_mask"])
    Wq = np.asarray(inputs["Wq"], np.float32)
    bq = np.asarray(inputs["bq"], np.float32)
    Wk = np.asarray(inputs["Wk"], np.float32)
    bk = np.asarray(inputs["bk"], np.float32)
    Wv = np.asarray(inputs["Wv"], np.float32)
    bv = np.asarray(inputs["bv"], np.float32)
    Wo = np.asarray(inputs["Wo"], np.float32)
    bo = np.asarray(inputs["bo"], np.float32)
    gamma = np.asarray(inputs["gamma"], np.float32)
    beta = np.asarray(inputs["beta"], np.float32)

    scale = 1.0 / np.sqrt(DH).astype(np.float32)
    wq_flat = np.ascontiguousarray(
        (Wq * scale).transpose(1, 0, 2).reshape(D, D))
    wk_flat = np.ascontiguousarray(Wk.transpose(1, 0, 2).reshape(D, D))
    bq_s = (bq * scale).reshape(D)
    bk_s = bk.reshape(D)

    wv_aug = np.zeros((D, NQUAD * 260), np.float32)
    bv_aug = np.zeros((1, NQUAD * 260), np.float32)
    for h in range(H):
        q, l = divmod(h, 4)
        base = q * 260 + l * 65
        wv_aug[:, base:base + 64] = Wv[h]
        bv_aug[0, base:base + 64] = bv[h]
        bv_aug[0, base + 64] = 1.0

    bqk = np.zeros((128, 2 * NPAIR), np.float32)
    for p in range(NPAIR):
        bqk[:, p] = bq_s[p * 128:(p + 1) * 128]
        bqk[:, NPAIR + p] = bk_s[p * 128:(p + 1) * 128]

    sel = np.zeros((128, 256), np.float32)
    for u in range(4):
        sel[32 * u, u * 64:(u + 1) * 64] = 1.0

    shared = {
        "wq": wq_flat, "wk": wk_flat, "wv": wv_aug,
        "wo": np.ascontiguousarray(Wo),
        "bqk": bqk, "bv": bv_aug,
        "gamma": gamma.reshape(1, D), "beta": beta.reshape(1, D),
        "bo": bo.reshape(1, D),
        "sel": sel,
        "onesr": np.ones((1, 128), np.float32),
        "bor": bo.reshape(1, D).copy(),
    }
    in_maps = []
    for b in range(B):
        mb = np.where(mask[b], 0.0, NEG_MASK).astype(np.float32)
        in_maps.append({
            **shared,
            "xt": np.ascontiguousarray(x[b].T),
            "maskb": np.ascontiguousarray(mb.reshape(SBLK, 128).T),
        })
    return in_maps


def _get_program():
    global _PROGRAM
    if _PROGRAM is None:
        _PROGRAM = _build_program()
    return _PROGRAM


def kernel(**inputs):
    from concourse.bass_utils import run_bass_kernel_spmd

    nc = _get_program()
    in_maps = _host_inputs(inputs)
    res = run_bass_kernel_spmd(nc, in_maps, list(range(B)))
    return np.stack([res.results[b]["out"] for b in range(B)], axis=0)


if __name__ == "__main__":
    rng = np.random.default_rng(0)
    demo = {
        "input_tensor": rng.standard_normal((B, S, D)).astype(np.float32),
        "attention_mask": np.ones((B, S), bool),
        "Wq": rng.standard_normal((H, D, DH)).astype(np.float32) * 0.03,
        "bq": rng.standard_normal((H, DH)).astype(np.float32) * 0.03,
        "Wk": rng.standard_normal((H, D, DH)).astype(np.float32) * 0.03,
        "bk": rng.standard_normal((H, DH)).astype(np.float32) * 0.03,
        "Wv": rng.standard_normal((H, D, DH)).astype(np.float32) * 0.03,
        "bv": rng.standard_normal((H, DH)).astype(np.float32) * 0.03,
        "Wo": rng.standard_normal((D, D)).astype(np.float32) * 0.03,
        "bo": rng.standard_normal((D,)).astype(np.float32) * 0.03,
        "gamma": np.ones((D,), np.float32),
        "beta": np.zeros((D,), np.float32),
    }
    out = kernel(**demo)
    print("kernel ran, out shape", out.shape, "finite:", np.isfinite(out).all())


# revision 37
# speedup vs baseline: 1.1032x; 1.1032x over previous
"""Multi-head attention + layernorm Bass kernel for Trainium2, 8 cores.

Problem: B=8, S=1024, D=768, H=12 heads x DH=64, key-padding mask, softmax,
output projection, layernorm.  Sharding: pure data parallelism — one batch
element per NeuronCore, no collectives.

Layout strategy (per core, all matmuls in float32r = 1 cyc/row at N>=256):
  - host passes x^T [768, 1024] so the d-contraction lives on partitions.
  - q^T/k^T built pair-of-heads-stacked: psum [128, 1024] = two heads' [64, S].
  - scores^T computed per key-chunk j with two concurrent row-tiled K=64
    matmuls (heads at partitions 0-63 / 64-127).
  - exp via ACT directly from PSUM (+ key mask as the per-partition bias).
  - ctx^T = sum_j v_aug[j]^T-style matmul with a ones column appended to V so
    each head's softmax denominators fall out of the same matmuls (row 64).
  - denominators: DVE reciprocal -> K=1 matmul broadcast -> DVE multiply while
    copying PSUM->SBUF (head B shifted to partitions 64-127).
  - out projection row-tiled over head pairs; layernorm via bn_stats/bn_aggr.
"""

import numpy as np

B, S, D, H, DH = 8, 1024, 768, 12, 64
NPAIR, NQUAD = H // 2, H // 4
SBLK = S // 128      # 8 key/row chunks
DCH = D // 128       # 6 contraction chunks
LN_EPS = 1e-5
NEG_MASK = -30.0

_PROGRAM = None


def _build_program():
    import concourse.bass as bass
    from concourse import bacc
    import concourse.tile as tile
    import concourse.mybir as mybir
    from contextlib import ExitStack

    F32 = mybir.dt.float32
    F32R = mybir.dt.float32r
    AF = mybir.ActivationFunctionType

    nc = bacc.Bacc("TRN2", target_bir_lowering=False)

    xt_d = nc.dram_tensor("xt", [D, S], F32R, kind="ExternalInput")
    wq_d = nc.dram_tensor("wq", [D, D], F32R, kind="ExternalInput")
    wk_d = nc.dram_tensor("wk", [D, D], F32R, kind="ExternalInput")
    wv_d = nc.dram_tensor("wv", [D, NQUAD * 260], F32R, kind="ExternalInput")
    wo_d = nc.dram_tensor("wo", [D, D], F32R, kind="ExternalInput")
    bqk_d = nc.dram_tensor("bqk", [128, 2 * NPAIR], F32, kind="ExternalInput")
    bv_d = nc.dram_tensor("bv", [1, NQUAD * 260], F32, kind="ExternalInput")
    maskb_d = nc.dram_tensor("maskb", [128, SBLK], F32, kind="ExternalInput")
    gamma_d = nc.dram_tensor("gamma", [1, D], F32, kind="ExternalInput")
    beta_d = nc.dram_tensor("beta", [1, D], F32, kind="ExternalInput")
    bo_d = nc.dram_tensor("bo", [1, D], F32, kind="ExternalInput")
    sel_d = nc.dram_tensor("sel", [128, 256], F32R, kind="ExternalInput")
    onesr_d = nc.dram_tensor("onesr", [1, 128], F32R, kind="ExternalInput")
    bor_d = nc.dram_tensor("bor", [1, D], F32R, kind="ExternalInput")
    out_d = nc.dram_tensor("out", [S, D], F32, kind="ExternalOutput")

    with tile.TileContext(nc) as tc, ExitStack() as ctx:
        const = ctx.enter_context(tc.tile_pool(name="const", bufs=1))
        xt_p = ctx.enter_context(tc.tile_pool(name="xt_p", bufs=1))
        w_p = ctx.enter_context(tc.tile_pool(name="w_p", bufs=1))
        qk_p = ctx.enter_context(tc.tile_pool(name="qk_p", bufs=1))
        v_p = ctx.enter_context(tc.tile_pool(name="v_p", bufs=1))
        e_p = ctx.enter_context(tc.tile_pool(name="e_p", bufs=1))
        cx_p = ctx.enter_context(tc.tile_pool(name="cx_p", bufs=1))
        z_p = ctx.enter_context(tc.tile_pool(name="z_p", bufs=1))
        # 8 PSUM banks: "big" = 3 x [128,1024] slots (q/k proj, scores, out
        # proj), "small" = 2 x [128,512] slots (v proj, ctx accum, denom bcast)
        ps = ctx.enter_context(tc.tile_pool(name="ps", bufs=1, space="PSUM"))

        # ---- constants ----
        bqk_t = const.tile([128, 2 * NPAIR], F32)
        nc.sync.dma_start(out=bqk_t, in_=bqk_d[:, :])
        bv_t = const.tile([128, NQUAD * 260], F32)
        nc.sync.dma_start(out=bv_t, in_=bv_d[0:1, :].to_broadcast([128, NQUAD * 260]))
        mask_t = const.tile([128, SBLK], F32)
        nc.sync.dma_start(out=mask_t, in_=maskb_d[:, :])
        gamma_t = const.tile([128, D], F32)
        nc.sync.dma_start(out=gamma_t, in_=gamma_d[0:1, :].to_broadcast([128, D]))
        beta_t = const.tile([128, D], F32)
        nc.sync.dma_start(out=beta_t, in_=beta_d[0:1, :].to_broadcast([128, D]))
        bo_t = const.tile([128, D], F32)
        nc.sync.dma_start(out=bo_t, in_=bo_d[0:1, :].to_broadcast([128, D]))
        sel_t = const.tile([128, 256], F32R)
        nc.sync.dma_start(out=sel_t, in_=sel_d[:, :])
        onesr_t = const.tile([1, 128], F32R)
        nc.sync.dma_start(out=onesr_t, in_=onesr_d[:, :])
        bor_t = const.tile([1, D], F32R)
        nc.sync.dma_start(out=bor_t, in_=bor_d[:, :])
        eps_t = const.tile([128, 1], F32)
        nc.vector.memset(eps_t, LN_EPS)

        xt = []
        for c in range(DCH):
            xt_t = xt_p.tile([128, S], F32R, name=f"xt{c}")
            nc.sync.dma_start(out=xt_t, in_=xt_d[c * 128:(c + 1) * 128, :])
            xt.append(xt_t)

        # ---- v projections, per quad of heads (N=260 keeps f32r fast) ----
        v_sb = {}   # (quad, sblk) -> [128, 260] f32r

        def emit_v_quad(q):
            wvq = w_p.tile([128, DCH, 260], F32R, name="wvq", bufs=3)
            nc.sync.dma_start(
                out=wvq,
                in_=wv_d[:, q * 260:(q + 1) * 260].rearrange(
                    "(c p) n -> p c n", p=128))
            wv_t = [wvq[:, c, :] for c in range(DCH)]
            for s in range(SBLK):
                psv = ps.tile([128, 260], F32, name="psv", tag="proj", bufs=1,
                              padded_shape=[128, 1024])
                for c in range(DCH):
                    nc.tensor.matmul(psv, xt[c][:, s * 128:(s + 1) * 128], wv_t[c],
                                     start=(c == 0), stop=(c == DCH - 1))
                vt = v_p.tile([128, 260], F32R, name="v_sb", bufs=3 * SBLK)
                nc.vector.tensor_add(out=vt, in0=psv,
                                     in1=bv_t[:, q * 260:(q + 1) * 260])
                v_sb[(q, s)] = vt

        # ---- per pair: q/k projections then attention ----
        ctxt = []   # per pair [128, 1024] f32r normalized ctx^T (B shifted)

        for q in range(NQUAD):
            emit_v_quad(q)

        for p in range(NPAIR):
            wqp = w_p.tile([128, DCH, 128], F32R, name="wqp", bufs=4)
            nc.sync.dma_start(
                out=wqp,
                in_=wq_d[:, p * 128:(p + 1) * 128].rearrange(
                    "(c p2) n -> p2 c n", p2=128))
            wq_t = [wqp[:, c, :] for c in range(DCH)]
            wkp = w_p.tile([128, DCH, 128], F32R, name="wkp", bufs=4)
            nc.sync.dma_start(
                out=wkp,
                in_=wk_d[:, p * 128:(p + 1) * 128].rearrange(
                    "(c p2) n -> p2 c n", p2=128))
            wk_t = [wkp[:, c, :] for c in range(DCH)]

            psq = ps.tile([128, S], F32, name="psq", tag="proj", bufs=1)
            for half in range(2):
                for c in range(DCH):
                    nc.tensor.matmul(
                        psq[:, half * 512:(half + 1) * 512], wq_t[c],
                        xt[c][:, half * 512:(half + 1) * 512],
                        start=(c == 0), stop=(c == DCH - 1))
            qt = qk_p.tile([128, S], F32R, name="qt_sb", bufs=3)
            nc.vector.tensor_scalar_add(out=qt, in0=psq, scalar1=bqk_t[:, p:p + 1])

            psk = ps.tile([128, S], F32, name="psk", tag="proj", bufs=1)
            for half in range(2):
                for c in range(DCH):
                    nc.tensor.matmul(
                        psk[:, half * 512:(half + 1) * 512], wk_t[c],
                        xt[c][:, half * 512:(half + 1) * 512],
                        start=(c == 0), stop=(c == DCH - 1))
            kt = qk_p.tile([128, S], F32R, name="kt_sb", bufs=3)
            nc.vector.tensor_scalar_add(out=kt, in0=psk,
                                        scalar1=bqk_t[:, NPAIR + p:NPAIR + p + 1])

            ct = cx_p.tile([128, S], F32R, name="ctxt", bufs=NPAIR)
            ctxt.append(ct)
            q, l0 = divmod(2 * p, 4)

            # softmax denominators for this pair: row 32*(2*idx + iblk) =
            # (head idx, query half iblk); engine writes need 32-aligned
            # partition bases.  Unused partitions memset to 1.0 so the
            # batched reciprocal stays finite.
            rpk = z_p.tile([128, 512], F32, name="rpk", bufs=3)
            nc.vector.memset(rpk, 1.0)
            for iblk in range(2):
                pcx = [ps.tile([65, 512], F32, name="pscx", tag="cx", bufs=2)
                       for _ in range(2)]
                for j in range(SBLK):
                    pst = ps.tile([128, 1024], F32, name="psst", tag="st",
                                  bufs=2)
                    nc.tensor.matmul(
                        pst[:, 0:512], kt[0:64, j * 128:(j + 1) * 128],
                        qt[0:64, iblk * 512:(iblk + 1) * 512],
                        start=True, stop=True, tile_position=(0, 0))
                    nc.tensor.matmul(
                        pst[:, 512:1024], kt[64:128, j * 128:(j + 1) * 128],
                        qt[64:128, iblk * 512:(iblk + 1) * 512],
                        start=True, stop=True, tile_position=(64, 0))
                    et = e_p.tile([128, 1024], F32R, name="expt", bufs=3)
                    nc.scalar.activation(et, pst, AF.Exp, bias=mask_t[:, j:j + 1])
                    for idx in range(2):
                        vsl = v_sb[(q, j)][:, (l0 + idx) * 65:(l0 + idx + 1) * 65]
                        nc.tensor.matmul(pcx[idx], vsl,
                                         et[:, idx * 512:(idx + 1) * 512],
                                         start=(j == 0), stop=(j == SBLK - 1))
                # move ctx (rows 0-63) and denominators (row 64) out of PSUM
                for idx in range(2):
                    u = 32 * (2 * idx + iblk)
                    nc.scalar.copy(out=rpk[u:u + 1, :],
                                   in_=pcx[idx][64:65, :])
                    nc.vector.tensor_copy(
                        out=ct[idx * 64:(idx + 1) * 64,
                               iblk * 512:(iblk + 1) * 512],
                        in_=pcx[idx][0:64, :])
            # one batched reciprocal per pair, then one-hot-selector matmuls
            # broadcast each row to [64, 512] and normalize in place
            rinv_p = z_p.tile([128, 512], F32R, name="rinv_p", bufs=2)
            with nc.allow_low_precision(reason="f32r softmax denom"):
                nc.vector.reciprocal(out=rinv_p, in_=rpk)
            for idx in range(2):
                for iblk in range(2):
                    u = 2 * idx + iblk
                    pbc = ps.tile([64, 512], F32, name="psbc", tag="cx",
                                  bufs=2)
                    nc.tensor.matmul(pbc, sel_t[:, u * 64:(u + 1) * 64],
                                     rinv_p, start=True, stop=True)
                    csl = ct[idx * 64:(idx + 1) * 64,
                             iblk * 512:(iblk + 1) * 512]
                    nc.vector.tensor_mul(out=csl, in0=csl, in1=pbc)

        # ---- output projection + layernorm, per row block ----
        # Wo loaded here so its DMAs don't compete with startup traffic
        woa = w_p.tile([128, DCH, D], F32R, name="woa", bufs=1)
        nc.sync.dma_start(out=woa,
                          in_=wo_d[:, :].rearrange("(c p) n -> p c n", p=128))
        wo_t = [woa[:, c, :] for c in range(DCH)]

        for s in range(SBLK):
            pso = ps.tile([128, D], F32, name="pso", tag="st", bufs=2,
                          padded_shape=[128, 1024])
            for d0, d1 in ((0, 512), (512, 768)):
                for p in range(NPAIR):
                    nc.tensor.matmul(
                        pso[:, d0:d1],
                        ctxt[p][:, s * 128:(s + 1) * 128],
                        wo_t[p][:, d0:d1],
                        start=(p == 0), stop=False)
                # + bo via a K=1 rank-one update: ones_col x bo_row
                nc.tensor.matmul(pso[:, d0:d1], onesr_t, bor_t[:, d0:d1],
                                 start=False, stop=True)
            stats = z_p.tile([128, 3, 6], F32, name="stats", bufs=2)
            for g in range(3):
                nc.vector.bn_stats(out=stats[:, g, :],
                                   in_=pso[:, g * 256:(g + 1) * 256])
            mv = z_p.tile([128, 2], F32, name="mv", bufs=2)
            nc.vector.bn_aggr(out=mv, in_=stats)
            stdv = z_p.tile([128, 1], F32, name="stdv", bufs=2)
            nc.scalar.activation(stdv, mv[:, 1:2], AF.Sqrt, bias=eps_t)
            rstd = z_p.tile([128, 1], F32, name="rstd", bufs=2)
            nc.vector.reciprocal(out=rstd, in_=stdv)
            nmr = z_p.tile([128, 1], F32, name="nmr", bufs=2)
            nc.vector.tensor_scalar(out=nmr, in0=mv[:, 0:1], scalar1=rstd,
                                    scalar2=-1.0, op0=mybir.AluOpType.mult,
                                    op1=mybir.AluOpType.mult)
            z = z_p.tile([128, D], F32, name="z_sb", bufs=2)
            nc.scalar.activation(z, pso, AF.Identity, bias=nmr, scale=rstd)
            nc.vector.tensor_mul(out=z, in0=z, in1=gamma_t)
            nc.vector.tensor_add(out=z, in0=z, in1=beta_t)
            nc.sync.dma_start(out=out_d[s * 128:(s + 1) * 128, :], in_=z)

    nc.compile()
    return nc


def _host_inputs(inputs):
    x = np.asarray(inputs["input_tensor"], np.float32)
    mask = np.asarray(inputs["attention_mask"])
    Wq = np.asarray(inputs["Wq"], np.float32)
    bq = np.asarray(inputs["bq"], np.float32)
    Wk = np.asarray(inputs["Wk"], np.float32)
    bk = np.asarray(inputs["bk"], np.float32)
    Wv = np.asarray(inputs["Wv"], np.float32)
    bv = np.asarray(inputs["bv"], np.float32)
    Wo = np.asarray(inputs["Wo"], np.float32)
    bo = np.asarray(inputs["bo"], np.float32)
    gamma = np.asarray(inputs["gamma"], np.float32)
    beta = np.asarray(inputs["beta"], np.float32)

    scale = 1.0 / np.sqrt(DH).astype(np.float32)
    wq_flat = np.ascontiguousarray(
        (Wq * scale).transpose(1, 0, 2).reshape(D, D))
    wk_flat = np.ascontiguousarray(Wk.transpose(1, 0, 2).reshape(D, D))
    bq_s = (bq * scale).reshape(D)
    bk_s = bk.reshape(D)

    wv_aug = np.zeros((D, NQUAD * 260), np.float32)
    bv_aug = np.zeros((1, NQUAD * 260), np.float32)
    for h in range(H):
        q, l = divmod(h, 4)
        base = q * 260 + l * 65
        wv_aug[:, base:base + 64] = Wv[h]
        bv_aug[0, base:base + 64] = bv[h]
        bv_aug[0, base + 64] = 1.0

    bqk = np.zeros((128, 2 * NPAIR), np.float32)
    for p in range(NPAIR):
        bqk[:, p] = bq_s[p * 128:(p + 1) * 128]
        bqk[:, NPAIR + p] = bk_s[p * 128:(p + 1) * 128]

    sel = np.zeros((128, 256), np.float32)
    for u in range(4):
        sel[32 * u, u * 64:(u + 1) * 64] = 1.0

    shared = {
        "wq": wq_flat, "wk": wk_flat, "wv": wv_aug,
        "wo": np.ascontiguousarray(Wo),
        "bqk": bqk, "bv": bv_aug,
        "gamma": gamma.reshape(1, D), "beta": beta.reshape(1, D),
        "bo": bo.reshape(1, D),
        "sel": sel,
        "onesr": np.ones((1, 128), np.float32),
        "bor": bo.reshape(1, D).copy(),
    }
    in_maps = []
    for b in range(B):
        mb = np.where(mask[b], 0.0, NEG_MASK).astype(np.float32)
        in_maps.append({
            **shared,
            "xt": np.ascontiguousarray(x[b].T),
            "maskb": np.ascontiguousarray(mb.reshape(SBLK, 128).T),
        })
    return in_maps


def _get_program():
    global _PROGRAM
    if _PROGRAM is None:
        _PROGRAM = _build_program()
    return _PROGRAM


def kernel(**inputs):
    from concourse.bass_utils import run_bass_kernel_spmd

    nc = _get_program()
    in_maps = _host_inputs(inputs)
    res = run_bass_kernel_spmd(nc, in_maps, list(range(B)))
    return np.stack([res.results[b]["out"] for b in range(B)], axis=0)


if __name__ == "__main__":
    rng = np.random.default_rng(0)
    demo = {
        "input_tensor": rng.standard_normal((B, S, D)).astype(np.float32),
        "attention_mask": np.ones((B, S), bool),
        "Wq": rng.standard_normal((H, D, DH)).astype(np.float32) * 0.03,
        "bq": rng.standard_normal((H, DH)).astype(np.float32) * 0.03,
        "Wk": rng.standard_normal((H, D, DH)).astype(np.float32) * 0.03,
        "bk": rng.standard_normal((H, DH)).astype(np.float32) * 0.03,
        "Wv": rng.standard_normal((H, D, DH)).astype(np.float32) * 0.03,
        "bv": rng.standard_normal((H, DH)).astype(np.float32) * 0.03,
        "Wo": rng.standard_normal((D, D)).astype(np.float32) * 0.03,
        "bo": rng.standard_normal((D,)).astype(np.float32) * 0.03,
        "gamma": np.ones((D,), np.float32),
        "beta": np.zeros((D,), np.float32),
    }
    out = kernel(**demo)
    print("kernel ran, out shape", out.shape, "finite:", np.isfinite(out).all())
